# revision 30
# baseline (speedup 1.0000x reference)
"""Multi-head causal attention + output projection on 8 Trainium2 cores.

Problem: B=4, T=2048, H=16, DQK=DV=64, E=1024, causal mask, fp32.

Sharding: core c -> batch b = c//2, head-group g = c%2 (8 heads each).
Each core computes full causal attention for its 8 heads and a partial
output projection (its heads' rows of W_o). Host sums the two partial
projections per batch and adds b_o.

Device algorithm (fp16 operands, fp32 PSUM accumulation), per head pair:
  scores^T(k,q) = K_h Q_h^T          (d on partitions; [k,q] layout)
  at = exp(scores^T * 1/8)           (ACT; fp16 out; no max-subtraction)
  causal: structural tile skipping + triangular mask on diagonal tiles
  ctx(q,65) = at^T [V_h | 1]         (transposed PV: q on partitions,
                                      col 64 = softmax denominator; PSUM)
  cs(q,(qsub,h,d)) = ctx * recip     (DVE per-partition scale, fp16)
  ctxT(hd, q) = dma_transpose(cs)    (XBAR 16x128-tile SBUF transpose)
  out(q,E) = ctxT.T @ W_o_rows       (lhsT=ctxT fp16, rhs=W_o fp16)
"""

import numpy as np

import concourse.bass as bass
import concourse.mybir as mybir
import concourse.tile as tile
from concourse import bacc
from concourse.bass_utils import run_bass_kernel_spmd

B, T, H, D, E = 4, 2048, 16, 64, 1024
HLOC = 8            # heads per core
NCORES = 8
TQ = 512            # q-block size
TK = 128            # k-tile size
NQB = T // TQ       # 4
NHP = HLOC // 2     # 4 head pairs
NKT = T // TK       # 16 k-tiles total
SCALE = 1.0 / np.sqrt(D)

F32 = mybir.dt.float32
F16 = mybir.dt.float16


def _build_nc():
    nc = bacc.Bacc("TRN2", target_bir_lowering=False, debug=False,
                   num_devices=NCORES, name="mha")
    qt_d = nc.dram_tensor("qt", [HLOC * D, T], F16, kind="ExternalInput")
    kt_d = nc.dram_tensor("kt", [HLOC * D, T], F16, kind="ExternalInput")
    vo_d = nc.dram_tensor("vo", [NHP, T, 2, 65], F16, kind="ExternalInput")
    wo_d = nc.dram_tensor("wo", [HLOC * D, E], F16, kind="ExternalInput")
    tri_d = nc.dram_tensor("tri", [TK, TK], F16, kind="ExternalInput")
    out_d = nc.dram_tensor("out", [T, E], F16, kind="ExternalOutput")

    EXP = mybir.ActivationFunctionType.Exp

    with tile.TileContext(nc) as tc:
        with (
            tc.tile_pool(name="const", bufs=1) as const_pool,
            tc.tile_pool(name="ctxT", bufs=1) as ctxT_pool,
            tc.tile_pool(name="qkt", bufs=1) as qkt_pool,
            tc.tile_pool(name="vsb", bufs=1) as v_pool,
            tc.tile_pool(name="attn", bufs=26) as attn_pool,
            tc.tile_pool(name="outsb", bufs=6) as out_pool,
            tc.tile_pool(name="csb", bufs=4) as cs_pool,
            tc.tile_pool(name="part", bufs=1) as part_pool,
            tc.tile_pool(name="rcs", bufs=4) as rc_pool,
            tc.tile_pool(name="scores", bufs=2, space="PSUM") as scores_pool,
            tc.tile_pool(name="ctxA", bufs=1, space="PSUM") as ctxA_pool,
            tc.tile_pool(name="ctxB", bufs=1, space="PSUM") as ctxB_pool,
            tc.tile_pool(name="proj", bufs=2, space="PSUM") as proj_pool,
        ):
            tri_sb = const_pool.tile([TK, TK], F16)

            # full-lhsT ctx^T for the output projection: [hd 128, hp, q]
            ctxT = ctxT_pool.tile([128, NHP, T], F16)

            kt_sbs, qt_sbs, v_sbs = [], [], []
            for hp in range(NHP):
                kt_sbs.append(qkt_pool.tile([128, T], F16, tag=f"kt{hp}",
                                            name="kt_sb"))
                qt_sbs.append(qkt_pool.tile([128, T], F16, tag=f"qt{hp}",
                                            name="qt_sb"))
                v_sbs.append(v_pool.tile([128, NKT, 2, 65], F16,
                                         tag=f"vP{hp}", name="vP"))
            # one whole-tensor DMA per (tensor, hp): DMA issue costs ~1.2us
            # of SP.SEQ+HWDGE each, so few big DMAs beat many chunks.
            # hp0's kt/qt split so the first QK can start ~2us earlier.
            for hp in range(NHP):
                hsl = slice(hp * 128, (hp + 1) * 128)
                vre = vo_d[hp].rearrange("(n p) h c -> p n h c", p=128)
                if hp == 0:
                    nc.scalar.dma_start(kt_sbs[0][:, 0:512],
                                        kt_d[hsl, 0:512])
                    nc.sync.dma_start(qt_sbs[0][:, T - TQ:T],
                                      qt_d[hsl, T - TQ:T])
                    nc.scalar.dma_start(tri_sb[:], tri_d[:])
                    nc.sync.dma_start(kt_sbs[0][:, 512:T], kt_d[hsl, 512:T])
                    nc.sync.dma_start(qt_sbs[0][:, 0:T - TQ],
                                      qt_d[hsl, 0:T - TQ])
                else:
                    nc.sync.dma_start(kt_sbs[hp][:], kt_d[hsl, :])
                    nc.sync.dma_start(qt_sbs[hp][:], qt_d[hsl, :])
                nc.sync.dma_start(v_sbs[hp][:], vre[:])
            wo_sb = const_pool.tile([128, 4, E], F16)
            nc.sync.dma_start(wo_sb[:], wo_d.rearrange("(n p) e -> p n e", p=128))

            COPY = mybir.ActivationFunctionType.Copy
            # m0-2 partial sums for the last qb's projections, computed
            # during the final attention step; the drain then only needs
            # one m3 matmul + add + store per output half
            partial = part_pool.tile([128, 8, 512], F32)

            def emit_partial(idx, qt_, eb):
                pp = proj_pool.tile([128, TQ], F32, tag="pp", name="pp")
                for m in range(NHP - 1):
                    nc.tensor.matmul(
                        pp[:, 0:512],
                        lhsT=ctxT[:, m, qt_ * 128:(qt_ + 1) * 128],
                        rhs=wo_sb[:, m, eb * 512:(eb + 1) * 512],
                        start=(m == 0), stop=(m == NHP - 2),
                    )
                nc.scalar.activation(partial[:, idx, :], pp[:, 0:512], COPY)

            def emit_final_proj(idx, qt_, eb):
                pp = proj_pool.tile([128, TQ], F32, tag="pp", name="pp")
                nc.tensor.matmul(
                    pp[:, 0:512],
                    lhsT=ctxT[:, NHP - 1, qt_ * 128:(qt_ + 1) * 128],
                    rhs=wo_sb[:, NHP - 1, eb * 512:(eb + 1) * 512],
                    start=True, stop=True,
                )
                ot = out_pool.tile([128, 512], F16, tag="oth", name="oth")
                nc.vector.tensor_add(ot[:], partial[:, idx, :], pp[:, 0:512])
                eng = nc.gpsimd if idx % 2 else nc.sync
                eng.dma_start(
                    out_d[qt_ * 128:(qt_ + 1) * 128, eb * 512:(eb + 1) * 512],
                    ot[:])

            def emit_proj_eb(qt_, eb):
                # half-proj: PE detour ~0.85us stays under ACT's 2-deep
                # score-buffer backlog, so exp never starves during a pop
                ot = out_pool.tile([128, 512], F16, tag="oth", name="oth")
                pp = proj_pool.tile([128, TQ], F32, tag="pp", name="pp")
                for m in range(NHP):
                    nc.tensor.matmul(
                        pp[:, 0:512],
                        lhsT=ctxT[:, m, qt_ * 128:(qt_ + 1) * 128],
                        rhs=wo_sb[:, m, eb * 512:(eb + 1) * 512],
                        start=(m == 0), stop=(m == NHP - 1),
                    )
                nc.vector.tensor_copy(ot[:], pp[:, 0:512])
                nc.gpsimd.dma_start(
                    out_d[qt_ * 128:(qt_ + 1) * 128, eb * 512:(eb + 1) * 512],
                    ot[:])

            def emit_proj(qt_, drain=False):
                ot = out_pool.tile([128, E], F16, tag="ot", name="ot")
                for eb in range(E // 512):
                    pp = proj_pool.tile([128, TQ], F32, tag="pp", name="pp")
                    for m in range(NHP):
                        nc.tensor.matmul(
                            pp[:, 0:512],
                            lhsT=ctxT[:, m, qt_ * 128:(qt_ + 1) * 128],
                            rhs=wo_sb[:, m, eb * 512:(eb + 1) * 512],
                            start=(m == 0), stop=(m == NHP - 1),
                        )
                    osl = slice(eb * 512, (eb + 1) * 512)
                    if drain:
                        # drain: ACT is idle there, DVE is the relay pacer
                        nc.scalar.activation(ot[:, osl], pp[:, 0:512], COPY)
                        nc.gpsimd.dma_start(
                            out_d[qt_ * 128:(qt_ + 1) * 128, osl], ot[:, osl])
                    else:
                        nc.vector.tensor_copy(ot[:, osl], pp[:, 0:512])
                if not drain:
                    # store via the (otherwise idle) gpsimd SWDGE queue so
                    # output stores never contend with the SP transpose queue
                    nc.gpsimd.dma_start(out_d[qt_ * 128:(qt_ + 1) * 128, :],
                                        ot[:])

            def emit_qk_exp(qb, hp, kk, nfull):
                q0 = max(kk - nfull, 0) * TK
                kt_sb, qt_sb = kt_sbs[hp], qt_sbs[hp]
                scr = scores_pool.tile([128, 2, TQ], F32, tag="scr", name="scr")
                at = attn_pool.tile([128, 2, TQ], F16, tag="attn", name="attn")
                for head in (0, 1):
                    dr = slice(head * D, head * D + D)
                    nc.tensor.matmul(
                        scr[:, head, q0:TQ],
                        lhsT=kt_sb[dr, kk * TK:(kk + 1) * TK],
                        rhs=qt_sb[dr, qb * TQ + q0:(qb + 1) * TQ],
                        start=True, stop=True,
                    )
                nc.scalar.activation(at[:, :, q0:TQ], scr[:, :, q0:TQ],
                                     EXP, scale=float(SCALE))
                if kk >= nfull:
                    # diagonal tile: mask k > q (both heads, broadcast tri)
                    trib = tri_sb[:].unsqueeze(1).broadcast_to([TK, 2, TK])
                    nc.vector.tensor_mul(at[:, :, q0:q0 + TK],
                                         at[:, :, q0:q0 + TK], trib)
                return at, q0

            def emit_pv_group(qb, hp, qsub, ats, ctx_ts):
                # transposed PV: out q on partitions, [V|1] moving (65 wide).
                # One full accumulation group per (head, qsub): PSUM banks
                # tolerate only one open accumulation group at a time, so
                # each region's start..stop sequence must not interleave
                # with another start in the same bank.
                nkq = 4 * qb + qsub + 1
                for kk in range(nkq):
                    for head in (0, 1):
                        nc.tensor.matmul(
                            ctx_ts[head][:, qsub, :],
                            lhsT=ats[kk][:, head, qsub * TK:(qsub + 1) * TK],
                            rhs=v_sbs[hp][:, kk, head, :],
                            start=(kk == 0), stop=(kk == nkq - 1),
                        )

            def make_epilogue(hp, qb, ctx_ts):
                def _ep():
                    # normalize per q row (partition-scalar) + fp16 evac,
                    # then one XBAR transpose into ctxT[:, hp, qb block]
                    rc = rc_pool.tile([128, 2, 4, 1], F32, tag="rc", name="rc")
                    cs = cs_pool.tile([128, 4, 2, 64], F16, tag="cs", name="cs")
                    for head in (0, 1):
                        ps = ctx_ts[head]
                        nc.vector.reciprocal(rc[:, head], ps[:, :, 64:65])
                        nc.vector.tensor_mul(
                            cs[:, :, head, :], ps[:, :, 0:64],
                            rc[:, head].broadcast_to([128, 4, 64]))
                    nc.sync.dma_start_transpose(
                        ctxT[:, hp, qb * TQ:(qb + 1) * TQ].rearrange(
                            "p (i j) -> p i j", j=128),
                        cs[:])
                return _ep

            # Software pipeline across (qb, hp) steps: each step's final PV
            # group, epilogue, and one proj emission are deferred into the
            # next step behind a WARM-deep QK/exp prefix, so ACT (the
            # bottleneck) always has exp work queued while PE runs the tail.
            pending_proj = []
            pending_tail = [None]
            WARM = 4

            def make_tail(qb, hp, ats, ctx_ts):
                def _tail():
                    emit_pv_group(qb, hp, 3, ats, ctx_ts)
                    make_epilogue(hp, qb, ctx_ts)()
                return _tail

            QB_ORDER = (3, 2, 1, 0)
            for qb in QB_ORDER:
                nk = (qb + 1) * (TQ // TK)
                nfull = nk - (TQ // TK)
                pop_kks = {3} if qb == 0 else {nk // 3, (2 * nk) // 3}

                for hp in range(NHP):
                    nw = min(WARM, nk - 1)
                    ats = []
                    for kk in range(nw):
                        at, _ = emit_qk_exp(qb, hp, kk, nfull)
                        ats.append(at)
                    if pending_tail[0] is not None:
                        pending_tail[0]()
                    ctx_ts = (ctxA_pool.tile([128, 4, 65], F32, tag="ctxA",
                                             name="ctxA"),
                              ctxB_pool.tile([128, 4, 65], F32, tag="ctxB",
                                             name="ctxB"))
                    for qsub in range(min(nw - nfull, 3)):
                        emit_pv_group(qb, hp, qsub, ats, ctx_ts)
                    for kk in range(nw, nk):
                        at, _ = emit_qk_exp(qb, hp, kk, nfull)
                        ats.append(at)
                        if kk in pop_kks and pending_proj:
                            emit_proj_eb(*pending_proj.pop(0))
                            if qb == 0 and pending_proj:
                                emit_proj_eb(*pending_proj.pop(0))
                        if nfull <= kk and kk - nfull < 3:
                            emit_pv_group(qb, hp, kk - nfull, ats, ctx_ts)
                    if qb == QB_ORDER[-1] and hp == NHP - 1:
                        for idx, (qt_, eb) in enumerate(
                                (q, e) for q in range(qb * 4, (qb + 1) * 4)
                                for e in range(2)):
                            emit_partial(idx, qt_, eb)
                    pending_tail[0] = make_tail(qb, hp, ats, ctx_ts)

                pending_proj.extend(
                    (qt_, eb) for qt_ in range(qb * 4, (qb + 1) * 4)
                    for eb in range(2))
            if pending_tail[0] is not None:
                pending_tail[0]()
            finals = sorted(set(pending_proj))
            for qt_, eb in [x for x in finals if x[0] >= 4]:
                emit_proj_eb(qt_, eb)  # older qb leftovers: no transpose dep
            finals = [x for x in finals if x[0] < 4]
            for idx, (qt_, eb) in enumerate(finals):
                emit_final_proj(idx, qt_, eb)

    nc.compile()
    return nc


_NC_CACHE = {}


def _get_nc():
    if "nc" not in _NC_CACHE:
        _NC_CACHE["nc"] = _build_nc()
    return _NC_CACHE["nc"]


def build_in_maps(Q, K, V, W_o):
    # transposed layout [k partitions, q free]: valid iff k <= q
    tri = np.triu(np.ones((TK, TK), dtype=np.float16))

    in_maps = []
    for c in range(NCORES):
        b, g = c // 2, c % 2
        hs = slice(g * HLOC * D, (g + 1) * HLOC * D)
        qt = np.ascontiguousarray(Q[b][:, hs].T).astype(np.float16)
        kt = np.ascontiguousarray(K[b][:, hs].T).astype(np.float16)
        vo = np.zeros((NHP, T, 2, 65), dtype=np.float16)
        for hp in range(NHP):
            for head in (0, 1):
                h0 = (g * HLOC + 2 * hp + head) * D
                vo[hp, :, head, 0:D] = V[b][:, h0:h0 + D]
            vo[hp, :, :, D] = 1.0  # ones column -> softmax denominators
        wo = np.ascontiguousarray(W_o[hs, :]).astype(np.float16)
        in_maps.append({"qt": qt, "kt": kt, "vo": vo, "wo": wo, "tri": tri})
    return in_maps


def _kernel_numpy(Q, K, V, mask, W_o, b_o):
    """Reference fallback for non-causal masks (never hit in practice)."""
    out = np.empty((B, T, E), dtype=np.float32)
    for b in range(B):
        q = Q[b].reshape(T, H, D).transpose(1, 0, 2)
        k = K[b].reshape(T, H, D).transpose(1, 0, 2)
        v = V[b].reshape(T, H, D).transpose(1, 0, 2)
        s = np.einsum("hqd,hkd->hqk", q, k) / np.sqrt(D)
        s = np.where(mask[b][None], -np.inf, s)
        a = np.exp(s - s.max(-1, keepdims=True))
        a /= a.sum(-1, keepdims=True)
        ctx = np.einsum("hqk,hkd->hqd", a, v).transpose(1, 0, 2).reshape(T, H * D)
        out[b] = ctx @ W_o + b_o
    return out


_CAUSAL = None


def _is_causal(mask):
    global _CAUSAL
    if _CAUSAL is None:
        _CAUSAL = np.triu(np.ones((T, T), dtype=bool), 1)
    m = np.asarray(mask)
    return m.shape == (B, T, T) and all(np.array_equal(m[b], _CAUSAL) for b in range(B))


def kernel(Q, K, V, mask, W_o, b_o):
    Q = np.asarray(Q, dtype=np.float32)
    K = np.asarray(K, dtype=np.float32)
    V = np.asarray(V, dtype=np.float32)
    W_o = np.asarray(W_o, dtype=np.float32)
    b_o = np.asarray(b_o, dtype=np.float32)

    if not _is_causal(mask):
        return _kernel_numpy(Q, K, V, np.asarray(mask, dtype=bool), W_o, b_o)

    in_maps = build_in_maps(Q, K, V, W_o)

    nc = _get_nc()
    res = run_bass_kernel_spmd(nc, in_maps, core_ids=list(range(NCORES)))
    _NC_CACHE["last_results"] = res

    out = np.empty((B, T, E), dtype=np.float32)
    for b in range(B):
        out[b] = (res.results[2 * b]["out"].astype(np.float32)
                  + res.results[2 * b + 1]["out"].astype(np.float32))
    out += b_o
    return out
